# revision 13
# baseline (speedup 1.0000x reference)
"""JointNet (RNN-T joint) Trainium2 Bass kernel.

out[b,t,u,c] = (enc @ W[:, :D].T)[b,t,c] + (dec @ W[:, D:].T)[b,u,c]

Shapes (hardcoded): B=4, T=512, U=100, D=512, C=1024; float32 in.
Full output (4, 512, 100, 1024) f32 = 839 MB; the device materializes it
in bf16 (420 MB, rel err ~4.5e-3 << 2e-2 gate) and the host upconverts.

Sharding: 8 cores; core k handles (b = k//2, t-half = k%2) -> a
(256, 100, 1024) output slab (~52 MB bf16) per core.

Per-core dataflow:
  host: prepack enc shard / dec[b] / W into partition-major layout
        ([d%128, chunk, free] flattened) so each input is ONE big DMA.
  PE:   enc_proj (2x [128,1024]) and dec_proj ([100,1024]) fp32 psum,
        drained to bf16 SBUF.
  main loop over u-groups of 10:
        dec_proj rows staged to partition 0 (SBUF->SBUF DMA); per u a
        K=1 ones-matmul (bf16, resident weights) broadcasts the row
        into a (128,1024) PSUM tile; ACT drains psum -> bf16 repl; DVE
        (packed 2x bf16 mode, ~0.7us per [128,1024]) adds enc_proj +
        repl into bf16 staging for both t-tiles; one 2.6 MB DMA per
        (t-tile, group) writes DRAM, alternating between the two HWDGE
        rings (sync/scalar) so transfers overlap.
  GpSimd is deliberately unused: measured ~2.3us per [128,1024]
  tensor_tensor AND it degrades concurrent DVE ops ~4x via SBUF
  contention.
  Roofline: 52.4 MB bf16 DMA write @ ~400 GB/s ~ 140 us/core; DVE adds
  ~140 us and ACT drains ~110 us run underneath.
"""

import numpy as np

import concourse.bass as bass
import concourse.bacc as bacc
import concourse.mybir as mybir
from concourse.bass_utils import run_bass_kernel_spmd
from concourse.tile import TileContext

B, T, U, D, C = 4, 512, 100, 512, 1024
TSH = T // 2          # t rows per core (two t-halves per batch)
P = 128               # partitions
NT = TSH // P         # t tiles per core = 2
KD = D // P           # contraction chunks per projection = 4
NB = C // 512         # psum banks per 1024-wide row = 2
UG = 10               # u rows per staged group
NG = U // UG          # groups = 10

_CACHE = {}


def _build_program():
    nc = bacc.Bacc(None, target_bir_lowering=False)
    f32 = mybir.dt.float32
    bf16 = mybir.dt.bfloat16

    enc_t = nc.dram_tensor("enc_t", [P, KD * TSH], f32, kind="ExternalInput")
    dec_t = nc.dram_tensor("dec_t", [P, KD * U], f32, kind="ExternalInput")
    w_enc = nc.dram_tensor("w_enc", [P, KD * C], f32, kind="ExternalInput")
    w_dec = nc.dram_tensor("w_dec", [P, KD * C], f32, kind="ExternalInput")
    out_sh = nc.dram_tensor("out_sh", [TSH, U, C], bf16, kind="ExternalOutput")

    with TileContext(nc) as tc, tc.tile_pool(name="persist", bufs=1) as pers:
        ones = pers.tile([1, P], bf16, tag="ones", name="ones")
        nc.vector.memset(ones, 1.0)

        # projections kept in bf16 (one rounding each; the later ones-matmul
        # and psum->bf16 drain of already-bf16 values are exact)
        enc_proj = [
            pers.tile([P, C], bf16, tag=f"enc_proj{tt}", name=f"enc_proj{tt}")
            for tt in range(NT)
        ]
        dec_proj = pers.tile([U, C], bf16, tag="dec_proj", name="dec_proj")

        # --- prologue: 3 bulk input DMAs, fp32 projections (dec first so
        # the main loop's flatten DMA can start early) ---
        with (
            tc.tile_pool(name="load", bufs=1) as loadp,
            tc.tile_pool(name="prol_psum", bufs=2, space="PSUM") as ppsum,
        ):
            dtile = loadp.tile([P, KD * U], f32, tag="dtile", name="dtile")
            nc.sync.dma_start(out=dtile, in_=dec_t[:, :])
            wdtile = loadp.tile([P, KD * C], f32, tag="wdtile", name="wdtile")
            nc.scalar.dma_start(out=wdtile, in_=w_dec[:, :])
            etile = loadp.tile([P, KD * TSH], f32, tag="etile", name="etile")
            nc.sync.dma_start(out=etile, in_=enc_t[:, :])
            wetile = loadp.tile([P, KD * C], f32, tag="wetile", name="wetile")
            nc.scalar.dma_start(out=wetile, in_=w_enc[:, :])

            # PE warm-up while input DMAs land: ~6us of dummy K=1 matmuls
            # keeps the HAM throttle window busy so the projections run at
            # the warm 2.4 GHz rate instead of cold/half-rate.
            warm = ppsum.tile([P, 512], f32, tag="warm", name="warm")
            for i in range(16):
                nc.tensor.matmul(
                    warm[:, :P],
                    ones,
                    ones,
                    start=(i == 0),
                    stop=(i == 15),
                )

            for cb in range(NB):
                pt = ppsum.tile([P, 512], f32, tag="prol")
                for dk in range(KD):
                    nc.tensor.matmul(
                        pt[:U],
                        dtile[:, dk * U : (dk + 1) * U],
                        wdtile[:, dk * C + cb * 512 : dk * C + (cb + 1) * 512],
                        start=(dk == 0),
                        stop=(dk == KD - 1),
                    )
                nc.vector.tensor_copy(
                    out=dec_proj[:, cb * 512 : (cb + 1) * 512], in_=pt[:U]
                )
            for tt in range(NT):
                for cb in range(NB):
                    pt = ppsum.tile([P, 512], f32, tag="prol")
                    for dk in range(KD):
                        nc.tensor.matmul(
                            pt,
                            etile[:, dk * TSH + tt * P : dk * TSH + (tt + 1) * P],
                            wetile[:, dk * C + cb * 512 : dk * C + (cb + 1) * 512],
                            start=(dk == 0),
                            stop=(dk == KD - 1),
                        )
                    nc.vector.tensor_copy(
                        out=enc_proj[tt][:, cb * 512 : (cb + 1) * 512], in_=pt
                    )

        # --- main loop: bf16 ones-matmul broadcast -> ACT psum drain ->
        # DVE packed-bf16 adds -> 2.6MB DMAs on alternating rings ---
        with (
            tc.tile_pool(name="decf", bufs=2) as flatp,
            tc.tile_pool(name="rep_psum", bufs=3, space="PSUM") as rpsum,
            tc.tile_pool(name="repl", bufs=3) as replp,
            tc.tile_pool(name="stage0", bufs=2) as st0,
            tc.tile_pool(name="stage1", bufs=2) as st1,
        ):
            stpools = [st0, st1]
            dma_engines = [nc.sync, nc.scalar]
            for g in range(NG):
                decf = flatp.tile([1, UG * C], bf16, tag="decf")
                nc.sync.dma_start(
                    out=decf, in_=dec_proj[g * UG : (g + 1) * UG, :]
                )
                stages = [
                    stpools[tt].tile([P, UG * C], bf16, tag=f"st{tt}", name=f"st{tt}_{g}")
                    for tt in range(NT)
                ]
                for uu in range(UG):
                    pr = rpsum.tile([P, C], f32, tag="rep")
                    for cb in range(NB):
                        off = uu * C + cb * 512
                        nc.tensor.matmul(
                            pr[:, cb * 512 : (cb + 1) * 512],
                            ones,
                            decf[0:1, off : off + 512],
                            start=True,
                            stop=True,
                        )
                    repl = replp.tile([P, C], bf16, tag="repl")
                    nc.scalar.copy(out=repl, in_=pr)
                    for tt in range(NT):
                        nc.vector.tensor_add(
                            out=stages[tt][:, uu * C : (uu + 1) * C],
                            in0=enc_proj[tt],
                            in1=repl,
                        )
                    # DMA chunking: middle groups ship one 2.62 MB DMA per
                    # t-tile (best rate); the first group ships 1.31 MB
                    # halves so the output stream starts ~5us earlier, and
                    # the last group ships halves so the final drain is
                    # ~7us shorter.
                    split = g == 0 or g == NG - 1
                    if (split and uu % (UG // 2) == UG // 2 - 1) or (
                        not split and uu == UG - 1
                    ):
                        u0 = uu + 1 - (UG // 2 if split else UG)
                        for tt in range(NT):
                            dma_engines[tt].dma_start(
                                out=out_sh[
                                    tt * P : (tt + 1) * P,
                                    g * UG + u0 : g * UG + uu + 1,
                                    :,
                                ],
                                in_=stages[tt][:, u0 * C : (uu + 1) * C],
                            )
    nc.finalize()
    return nc


def build_in_maps(enc, dec, w):
    """Prepack full inputs into the per-core partition-major DMA layout."""
    wt = w.T.reshape(2 * KD, P, C).transpose(1, 0, 2)  # (P, 2KD, C)
    we = np.ascontiguousarray(wt[:, :KD].reshape(P, KD * C))
    wd = np.ascontiguousarray(wt[:, KD:].reshape(P, KD * C))
    in_maps = []
    for core in range(8):
        b, th = core // 2, core % 2
        e = enc[b, th * TSH : (th + 1) * TSH, :].T  # (D, TSH)
        d = dec[b].T  # (D, U)
        in_maps.append(
            {
                "enc_t": np.ascontiguousarray(
                    e.reshape(KD, P, TSH).transpose(1, 0, 2).reshape(P, KD * TSH)
                ),
                "dec_t": np.ascontiguousarray(
                    d.reshape(KD, P, U).transpose(1, 0, 2).reshape(P, KD * U)
                ),
                "w_enc": we,
                "w_dec": wd,
            }
        )
    return in_maps


def kernel(encoder_outputs, decoder_outputs, W):
    enc = np.asarray(encoder_outputs, dtype=np.float32)
    dec = np.asarray(decoder_outputs, dtype=np.float32)
    w = np.asarray(W, dtype=np.float32)

    if "nc" not in _CACHE:
        _CACHE["nc"] = _build_program()
    nc = _CACHE["nc"]

    in_maps = build_in_maps(enc, dec, w)
    res = run_bass_kernel_spmd(nc, in_maps, list(range(8))).results

    out = np.empty((B, T, U, C), dtype=np.float32)
    for core in range(8):
        b, th = core // 2, core % 2
        slab = res[core]["out_sh"]  # (TSH, U, C) bfloat16
        u32 = slab.view(np.uint16).astype(np.uint32)
        u32 <<= 16
        out[b, th * TSH : (th + 1) * TSH] = u32.view(np.float32)
    return out


# revision 14
# speedup vs baseline: 1.0275x; 1.0275x over previous
"""JointNet (RNN-T joint) Trainium2 Bass kernel.

out[b,t,u,c] = (enc @ W[:, :D].T)[b,t,c] + (dec @ W[:, D:].T)[b,u,c]

Shapes (hardcoded): B=4, T=512, U=100, D=512, C=1024; float32 in.
Full output (4, 512, 100, 1024) f32 = 839 MB; the device materializes it
in bf16 (420 MB, rel err ~4.5e-3 << 2e-2 gate) and the host upconverts.

Sharding: 8 cores; core k handles (b = k//2, t-half = k%2) -> a
(256, 100, 1024) output slab (~52 MB bf16) per core.

Per-core dataflow:
  host: prepack enc shard / dec[b] / W into partition-major layout
        ([d%128, chunk, free] flattened) so each input is ONE big DMA.
  PE:   enc_proj (2x [128,1024]) and dec_proj ([100,1024]) fp32 psum,
        drained to bf16 SBUF.
  main loop over u-groups of 10:
        dec_proj rows staged to partition 0 (SBUF->SBUF DMA); per u a
        K=1 ones-matmul (bf16, resident weights) broadcasts the row
        into a (128,1024) PSUM tile; ACT drains psum -> bf16 repl; DVE
        (packed 2x bf16 mode, ~0.7us per [128,1024]) adds enc_proj +
        repl into bf16 staging for both t-tiles; one 2.6 MB DMA per
        (t-tile, group) writes DRAM, alternating between the two HWDGE
        rings (sync/scalar) so transfers overlap.
  GpSimd is deliberately unused: measured ~2.3us per [128,1024]
  tensor_tensor AND it degrades concurrent DVE ops ~4x via SBUF
  contention.
  Roofline: 52.4 MB bf16 DMA write @ ~400 GB/s ~ 140 us/core; DVE adds
  ~140 us and ACT drains ~110 us run underneath.
"""

import numpy as np

import concourse.bass as bass
import concourse.bacc as bacc
import concourse.mybir as mybir
from concourse.bass_utils import run_bass_kernel_spmd
from concourse.tile import TileContext

B, T, U, D, C = 4, 512, 100, 512, 1024
TSH = T // 2          # t rows per core (two t-halves per batch)
P = 128               # partitions
NT = TSH // P         # t tiles per core = 2
KD = D // P           # contraction chunks per projection = 4
NB = C // 512         # psum banks per 1024-wide row = 2
UG = 10               # u rows per staged group
NG = U // UG          # groups = 10

_CACHE = {}


def _build_program():
    nc = bacc.Bacc(None, target_bir_lowering=False)
    f32 = mybir.dt.float32
    bf16 = mybir.dt.bfloat16

    enc_t = nc.dram_tensor("enc_t", [P, KD * TSH], f32, kind="ExternalInput")
    dec_t = nc.dram_tensor("dec_t", [P, KD * U], f32, kind="ExternalInput")
    w_enc = nc.dram_tensor("w_enc", [P, KD * C], f32, kind="ExternalInput")
    w_dec = nc.dram_tensor("w_dec", [P, KD * C], f32, kind="ExternalInput")
    out_sh = nc.dram_tensor("out_sh", [TSH, U, C], bf16, kind="ExternalOutput")

    with TileContext(nc) as tc, tc.tile_pool(name="persist", bufs=1) as pers:
        ones = pers.tile([1, P], bf16, tag="ones", name="ones")
        nc.vector.memset(ones, 1.0)

        # projections kept in bf16 (one rounding each; the later ones-matmul
        # and psum->bf16 drain of already-bf16 values are exact)
        enc_proj = [
            pers.tile([P, C], bf16, tag=f"enc_proj{tt}", name=f"enc_proj{tt}")
            for tt in range(NT)
        ]
        dec_proj = pers.tile([U, C], bf16, tag="dec_proj", name="dec_proj")

        # --- prologue: 3 bulk input DMAs, fp32 projections (dec first so
        # the main loop's flatten DMA can start early) ---
        with (
            tc.tile_pool(name="load", bufs=1) as loadp,
            tc.tile_pool(name="prol_psum", bufs=2, space="PSUM") as ppsum,
        ):
            # input loads: W halves split into 1MB chunks spread across both
            # HWDGE rings so the projection matmuls start ~10us earlier
            dtile = loadp.tile([P, KD * U], f32, tag="dtile", name="dtile")
            nc.sync.dma_start(out=dtile, in_=dec_t[:, :])
            wdtile = loadp.tile([P, KD * C], f32, tag="wdtile", name="wdtile")
            nc.scalar.dma_start(out=wdtile[:, : 2 * C], in_=w_dec[:, : 2 * C])
            nc.sync.dma_start(out=wdtile[:, 2 * C :], in_=w_dec[:, 2 * C :])
            etile = loadp.tile([P, KD * TSH], f32, tag="etile", name="etile")
            nc.scalar.dma_start(out=etile, in_=enc_t[:, :])
            wetile = loadp.tile([P, KD * C], f32, tag="wetile", name="wetile")
            nc.sync.dma_start(out=wetile[:, : 2 * C], in_=w_enc[:, : 2 * C])
            nc.scalar.dma_start(out=wetile[:, 2 * C :], in_=w_enc[:, 2 * C :])

            # PE warm-up while input DMAs land: ~6us of dummy K=1 matmuls
            # keeps the HAM throttle window busy so the projections run at
            # the warm 2.4 GHz rate instead of cold/half-rate.
            warm = ppsum.tile([P, 512], f32, tag="warm", name="warm")
            for i in range(16):
                nc.tensor.matmul(
                    warm[:, :P],
                    ones,
                    ones,
                    start=(i == 0),
                    stop=(i == 15),
                )

            for cb in range(NB):
                pt = ppsum.tile([P, 512], f32, tag="prol")
                for dk in range(KD):
                    nc.tensor.matmul(
                        pt[:U],
                        dtile[:, dk * U : (dk + 1) * U],
                        wdtile[:, dk * C + cb * 512 : dk * C + (cb + 1) * 512],
                        start=(dk == 0),
                        stop=(dk == KD - 1),
                    )
                nc.vector.tensor_copy(
                    out=dec_proj[:, cb * 512 : (cb + 1) * 512], in_=pt[:U]
                )
            for tt in range(NT):
                for cb in range(NB):
                    pt = ppsum.tile([P, 512], f32, tag="prol")
                    for dk in range(KD):
                        nc.tensor.matmul(
                            pt,
                            etile[:, dk * TSH + tt * P : dk * TSH + (tt + 1) * P],
                            wetile[:, dk * C + cb * 512 : dk * C + (cb + 1) * 512],
                            start=(dk == 0),
                            stop=(dk == KD - 1),
                        )
                    nc.vector.tensor_copy(
                        out=enc_proj[tt][:, cb * 512 : (cb + 1) * 512], in_=pt
                    )

        # --- main loop: bf16 ones-matmul broadcast -> ACT psum drain ->
        # DVE packed-bf16 adds -> 2.6MB DMAs on alternating rings ---
        with (
            tc.tile_pool(name="decf", bufs=2) as flatp,
            tc.tile_pool(name="rep_psum", bufs=3, space="PSUM") as rpsum,
            tc.tile_pool(name="repl", bufs=3) as replp,
            tc.tile_pool(name="stage0", bufs=2) as st0,
            tc.tile_pool(name="stage1", bufs=2) as st1,
        ):
            stpools = [st0, st1]
            dma_engines = [nc.sync, nc.scalar]
            for g in range(NG):
                decf = flatp.tile([1, UG * C], bf16, tag="decf")
                nc.sync.dma_start(
                    out=decf, in_=dec_proj[g * UG : (g + 1) * UG, :]
                )
                stages = [
                    stpools[tt].tile([P, UG * C], bf16, tag=f"st{tt}", name=f"st{tt}_{g}")
                    for tt in range(NT)
                ]
                for uu in range(UG):
                    pr = rpsum.tile([P, C], f32, tag="rep")
                    for cb in range(NB):
                        off = uu * C + cb * 512
                        nc.tensor.matmul(
                            pr[:, cb * 512 : (cb + 1) * 512],
                            ones,
                            decf[0:1, off : off + 512],
                            start=True,
                            stop=True,
                        )
                    repl = replp.tile([P, C], bf16, tag="repl")
                    nc.scalar.copy(out=repl, in_=pr)
                    for tt in range(NT):
                        nc.vector.tensor_add(
                            out=stages[tt][:, uu * C : (uu + 1) * C],
                            in0=enc_proj[tt],
                            in1=repl,
                        )
                    # DMA chunking: middle groups ship one 2.62 MB DMA per
                    # t-tile (best rate); the first group ships 1.31 MB
                    # halves so the output stream starts ~5us earlier, and
                    # the last group ships halves so the final drain is
                    # ~7us shorter.
                    split = g == 0 or g == NG - 1
                    if (split and uu % (UG // 2) == UG // 2 - 1) or (
                        not split and uu == UG - 1
                    ):
                        u0 = uu + 1 - (UG // 2 if split else UG)
                        for tt in range(NT):
                            dma_engines[tt].dma_start(
                                out=out_sh[
                                    tt * P : (tt + 1) * P,
                                    g * UG + u0 : g * UG + uu + 1,
                                    :,
                                ],
                                in_=stages[tt][:, u0 * C : (uu + 1) * C],
                            )
    nc.finalize()
    return nc


def build_in_maps(enc, dec, w):
    """Prepack full inputs into the per-core partition-major DMA layout."""
    wt = w.T.reshape(2 * KD, P, C).transpose(1, 0, 2)  # (P, 2KD, C)
    we = np.ascontiguousarray(wt[:, :KD].reshape(P, KD * C))
    wd = np.ascontiguousarray(wt[:, KD:].reshape(P, KD * C))
    in_maps = []
    for core in range(8):
        b, th = core // 2, core % 2
        e = enc[b, th * TSH : (th + 1) * TSH, :].T  # (D, TSH)
        d = dec[b].T  # (D, U)
        in_maps.append(
            {
                "enc_t": np.ascontiguousarray(
                    e.reshape(KD, P, TSH).transpose(1, 0, 2).reshape(P, KD * TSH)
                ),
                "dec_t": np.ascontiguousarray(
                    d.reshape(KD, P, U).transpose(1, 0, 2).reshape(P, KD * U)
                ),
                "w_enc": we,
                "w_dec": wd,
            }
        )
    return in_maps


def kernel(encoder_outputs, decoder_outputs, W):
    enc = np.asarray(encoder_outputs, dtype=np.float32)
    dec = np.asarray(decoder_outputs, dtype=np.float32)
    w = np.asarray(W, dtype=np.float32)

    if "nc" not in _CACHE:
        _CACHE["nc"] = _build_program()
    nc = _CACHE["nc"]

    in_maps = build_in_maps(enc, dec, w)
    res = run_bass_kernel_spmd(nc, in_maps, list(range(8))).results

    out = np.empty((B, T, U, C), dtype=np.float32)
    for core in range(8):
        b, th = core // 2, core % 2
        slab = res[core]["out_sh"]  # (TSH, U, C) bfloat16
        u32 = slab.view(np.uint16).astype(np.uint32)
        u32 <<= 16
        out[b, th * TSH : (th + 1) * TSH] = u32.view(np.float32)
    return out


# revision 15
# speedup vs baseline: 1.1466x; 1.1160x over previous
"""JointNet (RNN-T joint) Trainium2 Bass kernel.

out[b,t,u,c] = (enc @ W[:, :D].T)[b,t,c] + (dec @ W[:, D:].T)[b,u,c]

Shapes (hardcoded): B=4, T=512, U=100, D=512, C=1024; float32 in.
Full output (4, 512, 100, 1024) f32 = 839 MB; the device materializes it
in bf16 (420 MB, rel err ~4.5e-3 << 2e-2 gate) and the host upconverts.

Sharding: 8 cores; core k handles (b = k//2, t-half = k%2) -> a
(256, 100, 1024) output slab (~52 MB bf16) per core.

Per-core dataflow:
  host: prepack enc shard / dec[b] / W into partition-major layout
        ([d%128, chunk, free] flattened) so each input is ONE big DMA.
  PE:   enc_proj (2x [128,1024]) and dec_proj ([100,1024]) fp32 psum,
        drained to bf16 SBUF.
  main loop over u-groups of 10:
        dec_proj rows staged to partition 0 (SBUF->SBUF DMA); per u a
        K=1 ones-matmul (bf16, resident weights) broadcasts the row
        into a (128,1024) PSUM tile; ACT drains psum -> bf16 repl; DVE
        (packed 2x bf16 mode, ~0.7us per [128,1024]) adds enc_proj +
        repl into bf16 staging for both t-tiles; one 2.6 MB DMA per
        (t-tile, group) writes DRAM, alternating between the two HWDGE
        rings (sync/scalar) so transfers overlap.
  GpSimd is deliberately unused: measured ~2.3us per [128,1024]
  tensor_tensor AND it degrades concurrent DVE ops ~4x via SBUF
  contention.
  Roofline: 52.4 MB bf16 DMA write @ ~400 GB/s ~ 140 us/core; DVE adds
  ~140 us and ACT drains ~110 us run underneath.
"""

import numpy as np

import concourse.bass as bass
import concourse.bacc as bacc
import concourse.mybir as mybir
from concourse.bass_utils import run_bass_kernel_spmd
from concourse.tile import TileContext

B, T, U, D, C = 4, 512, 100, 512, 1024
TSH = T // 2          # t rows per core (two t-halves per batch)
P = 128               # partitions
NT = TSH // P         # t tiles per core = 2
KD = D // P           # contraction chunks per projection = 4
NB = C // 512         # psum banks per 1024-wide row = 2
UG = 10               # u rows per staged group
NG = U // UG          # groups = 10

_CACHE = {}


def _build_program():
    nc = bacc.Bacc(None, target_bir_lowering=False)
    f32 = mybir.dt.float32
    bf16 = mybir.dt.bfloat16

    enc_t = nc.dram_tensor("enc_t", [P, KD * TSH], f32, kind="ExternalInput")
    dec_t = nc.dram_tensor("dec_t", [P, KD * U], f32, kind="ExternalInput")
    w_enc = nc.dram_tensor("w_enc", [P, KD * C], f32, kind="ExternalInput")
    w_dec = nc.dram_tensor("w_dec", [P, KD * C], f32, kind="ExternalInput")
    out_sh = nc.dram_tensor("out_sh", [TSH, U, C], bf16, kind="ExternalOutput")

    with TileContext(nc) as tc, tc.tile_pool(name="persist", bufs=1) as pers:
        ones = pers.tile([1, P], bf16, tag="ones", name="ones")
        nc.vector.memset(ones, 1.0)

        # projections kept in bf16 (one rounding each; the later ones-matmul
        # and psum->bf16 drain of already-bf16 values are exact)
        enc_proj = [
            pers.tile([P, C], bf16, tag=f"enc_proj{tt}", name=f"enc_proj{tt}")
            for tt in range(NT)
        ]
        dec_proj = pers.tile([U, C], bf16, tag="dec_proj", name="dec_proj")

        # --- prologue + main loop share one pool scope so the first
        # t-tile's output stream can start before the second t-tile's
        # projection is even computed ---
        with (
            tc.tile_pool(name="load", bufs=1) as loadp,
            tc.tile_pool(name="prol_psum", bufs=2, space="PSUM") as ppsum,
            tc.tile_pool(name="decf", bufs=2) as flatp,
            tc.tile_pool(name="rep_psum", bufs=2, space="PSUM") as rpsum,
            tc.tile_pool(name="repl", bufs=11) as replp,
            tc.tile_pool(name="stage0", bufs=2) as st0,
            tc.tile_pool(name="stage1", bufs=2) as st1,
        ):
            # input loads: W halves split into 1MB chunks spread across both
            # HWDGE rings so the projection matmuls start earlier
            dtile = loadp.tile([P, KD * U], f32, tag="dtile", name="dtile")
            nc.sync.dma_start(out=dtile, in_=dec_t[:, :])
            wdtile = loadp.tile([P, KD * C], f32, tag="wdtile", name="wdtile")
            nc.scalar.dma_start(out=wdtile[:, : 2 * C], in_=w_dec[:, : 2 * C])
            nc.sync.dma_start(out=wdtile[:, 2 * C :], in_=w_dec[:, 2 * C :])
            etile = loadp.tile([P, KD * TSH], f32, tag="etile", name="etile")
            nc.scalar.dma_start(out=etile, in_=enc_t[:, :])
            wetile = loadp.tile([P, KD * C], f32, tag="wetile", name="wetile")
            nc.sync.dma_start(out=wetile[:, : 2 * C], in_=w_enc[:, : 2 * C])
            nc.scalar.dma_start(out=wetile[:, 2 * C :], in_=w_enc[:, 2 * C :])

            # PE warm-up while input DMAs land: dummy K=1 matmuls keep the
            # HAM throttle window busy so projections don't run cold.
            warm = ppsum.tile([P, 512], f32, tag="prol", name="warm")
            for i in range(16):
                nc.tensor.matmul(
                    warm[:, :P],
                    ones,
                    ones,
                    start=(i == 0),
                    stop=(i == 15),
                )

            def dec_projection():
                for cb in range(NB):
                    pt = ppsum.tile([P, 512], f32, tag="prol", name=f"ptd{cb}")
                    for dk in range(KD):
                        nc.tensor.matmul(
                            pt[:U],
                            dtile[:, dk * U : (dk + 1) * U],
                            wdtile[:, dk * C + cb * 512 : dk * C + (cb + 1) * 512],
                            start=(dk == 0),
                            stop=(dk == KD - 1),
                        )
                    nc.vector.tensor_copy(
                        out=dec_proj[:, cb * 512 : (cb + 1) * 512], in_=pt[:U]
                    )

            def enc_projection(tt):
                for cb in range(NB):
                    pt = ppsum.tile([P, 512], f32, tag="prol", name=f"pte{tt}{cb}")
                    for dk in range(KD):
                        nc.tensor.matmul(
                            pt,
                            etile[:, dk * TSH + tt * P : dk * TSH + (tt + 1) * P],
                            wetile[:, dk * C + cb * 512 : dk * C + (cb + 1) * 512],
                            start=(dk == 0),
                            stop=(dk == KD - 1),
                        )
                    nc.vector.tensor_copy(
                        out=enc_proj[tt][:, cb * 512 : (cb + 1) * 512], in_=pt
                    )

            dec_projection()
            enc_projection(0)
            # enc_projection(1) is deferred until group 0 is in flight: its
            # 8 fp32 matmuls would otherwise delay the first output DMA.

            stpools = [st0, st1]
            dma_engines = [nc.sync, nc.scalar]

            def stage_dma(tt, stages, g, u0, u1):
                dma_engines[tt].dma_start(
                    out=out_sh[tt * P : (tt + 1) * P, g * UG + u0 : g * UG + u1, :],
                    in_=stages[tt][:, u0 * C : u1 * C],
                )

            for g in range(NG):
                decf = flatp.tile([1, UG * C], bf16, tag="decf")
                # flatten on the (otherwise idle) gpsimd DMA queue so it is
                # never stuck behind a 2.6MB stage write on the sync ring
                nc.gpsimd.dma_start(
                    out=decf, in_=dec_proj[g * UG : (g + 1) * UG, :]
                )
                stages = [
                    stpools[tt].tile([P, UG * C], bf16, tag=f"st{tt}", name=f"st{tt}_{g}")
                    for tt in range(NT)
                ]
                g0_repls = []
                for uu in range(UG):
                    pr = rpsum.tile([P, C], f32, tag="rep")
                    for cb in range(NB):
                        off = uu * C + cb * 512
                        nc.tensor.matmul(
                            pr[:, cb * 512 : (cb + 1) * 512],
                            ones,
                            decf[0:1, off : off + 512],
                            start=True,
                            stop=True,
                        )
                    repl = replp.tile([P, C], bf16, tag="repl")
                    nc.scalar.copy(out=repl, in_=pr)
                    nc.vector.tensor_add(
                        out=stages[0][:, uu * C : (uu + 1) * C],
                        in0=enc_proj[0],
                        in1=repl,
                    )
                    if g == 0:
                        g0_repls.append(repl)
                    else:
                        nc.vector.tensor_add(
                            out=stages[1][:, uu * C : (uu + 1) * C],
                            in0=enc_proj[1],
                            in1=repl,
                        )
                    # DMA chunking: middle groups ship one 2.62 MB DMA per
                    # t-tile (best rate); first/last groups ship 1.31 MB
                    # halves (earlier stream start / shorter final drain).
                    split = g == 0 or g == NG - 1
                    if (split and uu % (UG // 2) == UG // 2 - 1) or (
                        not split and uu == UG - 1
                    ):
                        u0 = uu + 1 - (UG // 2 if split else UG)
                        stage_dma(0, stages, g, u0, uu + 1)
                        if g != 0:
                            stage_dma(1, stages, g, u0, uu + 1)
                if g == 0:
                    # group 0 is streaming t-tile 0; now compute the second
                    # enc projection and catch up t-tile 1.
                    enc_projection(1)
                    for uu in range(UG):
                        nc.vector.tensor_add(
                            out=stages[1][:, uu * C : (uu + 1) * C],
                            in0=enc_proj[1],
                            in1=g0_repls[uu],
                        )
                        if uu % (UG // 2) == UG // 2 - 1:
                            stage_dma(1, stages, g, uu + 1 - UG // 2, uu + 1)
    nc.finalize()
    return nc


def build_in_maps(enc, dec, w):
    """Prepack full inputs into the per-core partition-major DMA layout."""
    wt = w.T.reshape(2 * KD, P, C).transpose(1, 0, 2)  # (P, 2KD, C)
    we = np.ascontiguousarray(wt[:, :KD].reshape(P, KD * C))
    wd = np.ascontiguousarray(wt[:, KD:].reshape(P, KD * C))
    in_maps = []
    for core in range(8):
        b, th = core // 2, core % 2
        e = enc[b, th * TSH : (th + 1) * TSH, :].T  # (D, TSH)
        d = dec[b].T  # (D, U)
        in_maps.append(
            {
                "enc_t": np.ascontiguousarray(
                    e.reshape(KD, P, TSH).transpose(1, 0, 2).reshape(P, KD * TSH)
                ),
                "dec_t": np.ascontiguousarray(
                    d.reshape(KD, P, U).transpose(1, 0, 2).reshape(P, KD * U)
                ),
                "w_enc": we,
                "w_dec": wd,
            }
        )
    return in_maps


def kernel(encoder_outputs, decoder_outputs, W):
    enc = np.asarray(encoder_outputs, dtype=np.float32)
    dec = np.asarray(decoder_outputs, dtype=np.float32)
    w = np.asarray(W, dtype=np.float32)

    if "nc" not in _CACHE:
        _CACHE["nc"] = _build_program()
    nc = _CACHE["nc"]

    in_maps = build_in_maps(enc, dec, w)
    res = run_bass_kernel_spmd(nc, in_maps, list(range(8))).results

    out = np.empty((B, T, U, C), dtype=np.float32)
    for core in range(8):
        b, th = core // 2, core % 2
        slab = res[core]["out_sh"]  # (TSH, U, C) bfloat16
        u32 = slab.view(np.uint16).astype(np.uint32)
        u32 <<= 16
        out[b, th * TSH : (th + 1) * TSH] = u32.view(np.float32)
    return out
